# revision 35
# baseline (speedup 1.0000x reference)
"""Trainium2 Bass kernel for nn_BasicRecurrentEntityEncoder.

Data-parallel over batch B=256 across 8 NeuronCores (32 batches/core).
Per core, entity rows are laid out k-major: j = k*32 + b (K padded 30->32),
giving 1024 rows = 8 chunks x 128 partitions with b = p % 32 uniform in
every chunk.

v2 design: the 8 row-chunks are split into TWO independent groups (chunks
0-3 / 4-7) whose recurrences don't interact, so their per-step chains
software-pipeline across engines.  Per-step transposes (natural [j,d] ->
[d,j] for the PE) are done on the DMA xbar (dma_start_transpose), PE does
only productive matmuls.  All scalar-engine activations (exp for the gate
sigmoid, ln/exp for rsqrt, relu) live in ONE act-table set
(natural_log_exp_and_others), so no per-step ACT_TABLE_LOAD thrash.
Gated update is fused: upd = (relu(pn) * g) + h via one scalar_tensor_tensor
per chunk; squared-norm via tensor_tensor_reduce (accum = eps + sum(upd^2)).

Phase A (interleaved): indirect-DMA gather of bf16 embedding rows, on-chip
bag-of-words sums, encT / eW / EK / keys@V precomputes per group of 8 steps.
"""

import os
import numpy as np
import ml_dtypes

B, S, L, D, K, VOCAB = 256, 64, 16, 256, 30, 50000
NCORES = 8
BL = B // NCORES          # 32 batches per core
KH = 32                   # padded K
J = KH * BL               # 1024 rows per core
CH = 8                    # row chunks (128 partitions each)
NG = 2                    # row groups (4 chunks each)
CPG = CH // NG            # chunks per group
GRP = 8                   # gather groups
SPG = S // GRP            # steps per group
NEG = -60.0               # gate logit offset for masked sentences
EPS = 1e-12

LAST_EXEC_NS = None       # set when BASS_KERNEL_TRACE=1
NSTEPS = int(os.environ.get("BK_NSTEPS", str(S)))
SKIP_GATHER = os.environ.get("BK_SKIP_GATHER", "0") == "1"
NEWTON2 = os.environ.get("BK_NEWTON2", "0") == "1"

_bf16 = ml_dtypes.bfloat16


def _build_nc():
    import concourse.bacc as bacc
    import concourse.mybir as mybir
    from concourse import tile

    f32 = mybir.dt.float32
    bf16 = mybir.dt.bfloat16
    i32 = mybir.dt.int32
    MULT = mybir.AluOpType.mult
    ADD = mybir.AluOpType.add
    SUB = mybir.AluOpType.subtract
    ASR = mybir.AluOpType.arith_shift_right
    TANH = mybir.ActivationFunctionType.Tanh
    IDENT = mybir.ActivationFunctionType.Identity
    RELU = mybir.ActivationFunctionType.Relu
    QC = 0x5f3759df

    nc = bacc.Bacc("TRN2", target_bir_lowering=False, debug=False,
                   num_devices=NCORES)

    # ---- DRAM parameters -------------------------------------------------
    emb = nc.dram_tensor("emb", [8192, 4 * D], bf16, kind="ExternalInput")
    idx_d = nc.dram_tensor("idx", [128, 512], mybir.dt.int16, kind="ExternalInput")
    keysT_d = nc.dram_tensor("keysT", [128, 2, J], bf16, kind="ExternalInput")
    u_d = nc.dram_tensor("u", [128, 2, D], bf16, kind="ExternalInput")
    v_d = nc.dram_tensor("v", [128, 2, D], bf16, kind="ExternalInput")
    w_d = nc.dram_tensor("w", [128, 2, D], bf16, kind="ExternalInput")
    bias_d = nc.dram_tensor("bias", [128, S], f32, kind="ExternalInput")
    selsum_d = nc.dram_tensor("selsum", [128, BL], bf16, kind="ExternalInput")
    selkm_d = nc.dram_tensor("selkm", [BL, 128], bf16, kind="ExternalInput")
    mdiag_d = nc.dram_tensor("mdiag", [128, BL], f32, kind="ExternalInput")
    ident_d = nc.dram_tensor("ident", [128, 128], bf16, kind="ExternalInput")
    y_d = nc.dram_tensor("y", [BL, K, D], f32, kind="ExternalOutput")

    with tile.TileContext(nc) as tc:
        ctxs = []

        def pool(name, bufs, space="SBUF"):
            p = tc.tile_pool(name=name, bufs=bufs, space=space)
            ctxs.append(p)
            return p.__enter__()

        persist = pool("persist", 1)
        gbuf = pool("gbuf", 2)
        ps_pn = pool("ps_pn", 1, "PSUM")        # 2 x [128, 1024] f32 (2 banks each)
        ps_gps = pool("ps_gps", 1, "PSUM")      # 2 x [128, 128] f32 (1 bank each)
        ps_pre = pool("ps_pre", 2, "PSUM")      # [128, 512] f32 slots (1 bank each)

        # ---- persistent SBUF tensors ------------------------------------
        idx_sb = persist.tile([128, 512], mybir.dt.int16, tag="idx")
        keysT = persist.tile([128, 2, J], bf16, tag="keysT")
        u_sb = persist.tile([128, 2, D], bf16, tag="u")
        v_sb = persist.tile([128, 2, D], bf16, tag="v")
        w_sb = persist.tile([128, 2, D], bf16, tag="w")
        bias_sb = persist.tile([128, S], f32, tag="bias")
        selsum = persist.tile([128, BL], bf16, tag="selsum")
        selkm = persist.tile([BL, 128], bf16, tag="selkm")
        mdiag = persist.tile([128, BL], f32, tag="mdiag")
        ident = persist.tile([128, 128], bf16, tag="ident")
        encT = persist.tile([128, 2, S * BL], bf16, tag="encT")
        ew_all = persist.tile([BL, S * D], bf16, tag="ew")
        ekm = persist.tile([128, CH, S], f32, tag="ekm")
        kv = persist.tile([128, CH, D], bf16, tag="kv")
        ek_scr = persist.tile([128, 2, SPG, BL], bf16, tag="ekscr")
        ekr = persist.tile([128, 2, SPG], f32, tag="ekred")
        hf32 = persist.tile([128, CH, D], f32, tag="hf32")

        # per-group state (hT is chunk-major: [p, lc, half, 128])
        h_nat, hT, tt, gmx, gpre, gp2, th, gg, ss, rr, dmb = (
            [], [], [], [], [], [], [], [], [], [], [])
        ny0, ny2, nt1 = [], [], []
        for g in range(NG):
            h_nat.append(persist.tile([128, CPG, D], bf16, tag=f"hnat{g}",
                                      name=f"hnat{g}"))
            hT.append(persist.tile([128, CPG, 2, 128], bf16, tag=f"hT{g}",
                                   name=f"hT{g}"))
            tt.append(persist.tile([128, CPG, D], bf16, tag=f"tt{g}",
                                   name=f"tt{g}"))
            gmx.append(persist.tile([128, CPG, BL], f32, tag=f"gmx{g}",
                                    name=f"gmx{g}"))
            gpre.append(persist.tile([128, CPG], f32, tag=f"gpre{g}",
                                     name=f"gpre{g}"))
            gp2.append(persist.tile([128, CPG], f32, tag=f"gp2{g}",
                                    name=f"gp2{g}"))
            th.append(persist.tile([128, CPG], f32, tag=f"th{g}",
                                   name=f"th{g}"))
            gg.append(persist.tile([128, CPG], f32, tag=f"gg{g}",
                                   name=f"gg{g}"))
            ss.append(persist.tile([128, CPG], f32, tag=f"ss{g}",
                                   name=f"ss{g}"))
            rr.append(persist.tile([128, CPG], f32, tag=f"rr{g}",
                                   name=f"rr{g}"))
            dmb.append(persist.tile([128, CPG, D], bf16, tag=f"dmb{g}",
                                    name=f"dmb{g}"))
            ny0.append(persist.tile([128, CPG], f32, tag=f"ny0{g}",
                                    name=f"ny0{g}"))
            ny2.append(persist.tile([128, CPG], f32, tag=f"ny2{g}",
                                    name=f"ny2{g}"))
            nt1.append(persist.tile([128, CPG], f32, tag=f"nt1{g}",
                                    name=f"nt1{g}"))
        qct = persist.tile([128, NG * CPG], i32, tag="qct")
        h05 = persist.tile([128, 1], f32, tag="h05")

        # ---- load parameters --------------------------------------------
        nc.sync.dma_start(out=idx_sb[:], in_=idx_d.ap())
        nc.sync.dma_start(out=keysT[:], in_=keysT_d.ap())
        nc.sync.dma_start(out=u_sb[:], in_=u_d.ap())
        nc.sync.dma_start(out=v_sb[:], in_=v_d.ap())
        nc.sync.dma_start(out=w_sb[:], in_=w_d.ap())
        nc.sync.dma_start(out=bias_sb[:], in_=bias_d.ap())
        nc.sync.dma_start(out=selsum[:], in_=selsum_d.ap())
        nc.sync.dma_start(out=selkm[:], in_=selkm_d.ap())
        nc.sync.dma_start(out=mdiag[:], in_=mdiag_d.ap())
        nc.sync.dma_start(out=ident[:], in_=ident_d.ap())

        nc.vector.memset(qct[:], QC)
        nc.vector.memset(h05[:], 0.5)
        for g in range(NG):
            nc.vector.memset(h_nat[g][:], 0.0)
            nc.vector.memset(hT[g][:], 0.0)

        # ========== gathers + per-group precompute ========================
        # gathers are split in 256-idx quarters so they don't monopolize
        # the gpsimd FIFO (which also runs the per-step squares)
        def alloc_raw():
            raw = gbuf.tile([128, 4 * SPG, D], bf16, tag="raw", name="raw")
            return raw

        def emit_gather_quarter(g, h, raw):
            nc.gpsimd.dma_gather(
                out_ap=raw[:].rearrange("p (q k) d -> p q (k d)",
                                        k=4)[:, 2 * h:2 * h + 2, :],
                in_ap=emb.ap(),
                idxs_ap=idx_sb[:, g * 64 + 16 * h:g * 64 + 16 * (h + 1)],
                num_idxs=256, num_idxs_reg=256, elem_size=4 * D)

        def emit_group_precompute(g, raw):
            # l-sum: raw[p, (s_in, l_hi), d] -> part[p, s_in, d]
            s02 = gbuf.tile([128, SPG, 2, D], bf16, tag="s02")
            r4 = raw[:].rearrange("p (s l) d -> p s l d", l=4)
            nc.vector.tensor_tensor(out=s02[:], in0=r4[:, :, 0:2, :],
                                    in1=r4[:, :, 2:4, :], op=ADD)
            part = gbuf.tile([128, SPG, D], bf16, tag="part")
            nc.vector.tensor_tensor(out=part[:], in0=s02[:, :, 0, :],
                                    in1=s02[:, :, 1, :], op=ADD)
            # encT[half][d, (s, b)] via PE: part.T @ selsum
            for half in range(2):
                etp = ps_pre.tile([128, SPG * BL], f32, tag="pre")
                for si in range(SPG):
                    nc.tensor.matmul(
                        out=etp[:, si * BL:(si + 1) * BL],
                        lhsT=part[:, si, half * 128:(half + 1) * 128],
                        rhs=selsum[:], start=(si == 0), stop=(si == SPG - 1))
                nc.vector.tensor_copy(
                    out=encT[:, half, g * SPG * BL:(g + 1) * SPG * BL],
                    in_=etp[:])
            # eW[b, (s, d)] for this group, si-pairs in 1-bank psum tiles
            for rp in range(SPG // 2):
                ewp = ps_pre.tile([BL, 2 * D], f32, tag="pre")
                for k in range(2):
                    s = g * SPG + rp * 2 + k
                    for half in range(2):
                        nc.tensor.matmul(
                            out=ewp[:, k * D:(k + 1) * D],
                            lhsT=encT[:, half, s * BL:(s + 1) * BL],
                            rhs=w_sb[:, half, :],
                            start=(k == 0 and half == 0),
                            stop=(k == 1 and half == 1))
                s0 = g * SPG + rp * 2
                nc.scalar.copy(out=ew_all[:, s0 * D:(s0 + 2) * D], in_=ewp[:])
            # EK for this group -> ekm[:, :, 8g:8g+8], chunk-pairs
            for rp in range(CH // 2):
                ekx = ps_pre.tile([128, 2, SPG * BL], f32, tag="pre")
                for k in range(2):
                    c = rp * 2 + k
                    for half in range(2):
                        nc.tensor.matmul(
                            out=ekx[:, k, :],
                            lhsT=keysT[:, half, c * 128:(c + 1) * 128],
                            rhs=encT[:, half, g * SPG * BL:(g + 1) * SPG * BL],
                            start=(half == 0), stop=(half == 1))
                nc.vector.tensor_tensor(
                    out=ek_scr[:],
                    in0=ekx[:].rearrange("p k (s b) -> p k s b", b=BL),
                    in1=mdiag[:].unsqueeze(1).unsqueeze(1).broadcast_to(
                        [128, 2, SPG, BL]),
                    op=MULT)
                nc.vector.tensor_reduce(
                    out=ekr[:], in_=ek_scr[:], axis=mybir.AxisListType.X,
                    op=ADD)
                nc.vector.tensor_tensor(
                    out=ekm[:, rp * 2:rp * 2 + 2, g * SPG:(g + 1) * SPG],
                    in0=ekr[:],
                    in1=bias_sb[:, g * SPG:(g + 1) * SPG].unsqueeze(1)
                        .broadcast_to([128, 2, SPG]),
                    op=ADD)

        # kV[p, c, d] = keys @ V (needs only keysT)
        for c in range(CH):
            kvp = ps_pre.tile([128, D], f32, tag="pre")
            for half in range(2):
                nc.tensor.matmul(out=kvp[:],
                                 lhsT=keysT[:, half, c * 128:(c + 1) * 128],
                                 rhs=v_sb[:, half, :],
                                 start=(half == 0), stop=(half == 1))
            nc.vector.tensor_copy(out=kv[:, c, :], in_=kvp[:])

        def scan_front(s, grp):
            c0 = grp * CPG
            hTg, hn, ttg = hT[grp], h_nat[grp], tt[grp]
            pn = ps_pn.tile([128, CPG * D], f32, tag=f"pn{grp}")
            gps = ps_gps.tile([128, CPG * BL], f32, tag=f"gps{grp}")
            # bias adds: kV via identity (bank pairs), eW via selection
            for bp in range(CPG // 2):
                nc.tensor.matmul(out=pn[:, bp * 2 * D:(bp + 1) * 2 * D],
                                 lhsT=ident[:],
                                 rhs=kv[:, c0 + bp * 2:c0 + bp * 2 + 2, :],
                                 start=True, stop=False)
            for bp in range(CPG // 2):
                nc.tensor.matmul(
                    out=pn[:, bp * 2 * D:(bp + 1) * 2 * D], lhsT=selkm[:],
                    rhs=ew_all[:, s * D:(s + 1) * D].unsqueeze(1)
                        .broadcast_to([BL, 2, D]),
                    start=False, stop=False)
            # gate MMs first (gate postprocess overlaps the h@U MMs)
            for half in range(2):
                for lc in range(CPG):
                    nc.tensor.matmul(out=gps[:, lc * BL:(lc + 1) * BL],
                                     lhsT=hTg[:, lc, half, :],
                                     rhs=encT[:, half, s * BL:(s + 1) * BL],
                                     start=(half == 0 and lc == 0),
                                     stop=(half == 1 and lc == CPG - 1))
            for half in range(2):
                for lc in range(CPG):
                    nc.tensor.matmul(out=pn[:, lc * D:(lc + 1) * D],
                                     lhsT=hTg[:, lc, half, :],
                                     rhs=u_sb[:, half, :], start=False,
                                     stop=(half == 1 and lc % 2 == 1))
            # gate: mask diag, reduce, +EK, sigmoid via tanh (g = .5*th+.5)
            nc.vector.tensor_tensor(
                out=gmx[grp][:],
                in0=gps[:].rearrange("p (c b) -> p c b", b=BL),
                in1=mdiag[:].unsqueeze(1).broadcast_to([128, CPG, BL]),
                op=MULT)
            nc.vector.tensor_reduce(out=gpre[grp][:], in_=gmx[grp][:],
                                    axis=mybir.AxisListType.X, op=ADD)
            nc.vector.tensor_tensor(out=gp2[grp][:], in0=gpre[grp][:],
                                    in1=ekm[:, c0:c0 + CPG, s], op=ADD)
            nc.scalar.activation(th[grp][:], gp2[grp][:], TANH, scale=0.5)
            nc.vector.tensor_scalar(out=gg[grp][:], in0=th[grp][:],
                                    scalar1=0.5, scalar2=0.5,
                                    op0=MULT, op1=ADD)
            # t_g = g*relu(pn) per chunk on ACT (scale multiplies before
            # relu; g>0 so relu(pn*g) == g*relu(pn))
            for lc in range(CPG):
                nc.scalar.activation(ttg[:, lc, :],
                                     pn[:, lc * D:(lc + 1) * D], RELU,
                                     scale=gg[grp][:, lc:lc + 1])
            # upd = t_g + h; squares on gpsimd; reduce on DVE (per half)
            for hh in range(2):
                sl = slice(2 * hh, 2 * hh + 2)
                nc.vector.tensor_tensor(
                    out=ttg[:, sl, :], in0=ttg[:, sl, :],
                    in1=hn[:, sl, :], op=ADD)
                nc.gpsimd.tensor_tensor(out=dmb[grp][:, sl, :],
                                        in0=ttg[:, sl, :],
                                        in1=ttg[:, sl, :], op=MULT)
                nc.vector.tensor_reduce(out=ss[grp][:, sl],
                                        in_=dmb[grp][:, sl, :],
                                        axis=mybir.AxisListType.X, op=ADD)
        def scan_back(s, grp):
            last = (s == NSTEPS - 1)
            c0 = grp * CPG
            hTg, hn, ttg = hT[grp], h_nat[grp], tt[grp]
            # r = rsqrt(ss) via quake seed + 1 Newton iter (all on DVE;
            # ss=0 rows stay finite: r=1.5*y0, upd=0 -> h=0)
            y0, y2, t1 = ny0[grp], ny2[grp], nt1[grp]
            if NEWTON2:
                # eps keeps iter-2 finite for all-zero rows
                nc.vector.tensor_scalar(out=ss[grp][:], in0=ss[grp][:],
                                        scalar1=EPS, scalar2=None, op0=ADD)
            nc.vector.tensor_scalar(
                out=y2[:].bitcast(i32), in0=ss[grp][:].bitcast(i32),
                scalar1=1, scalar2=None, op0=ASR)
            nc.vector.tensor_tensor(
                out=y0[:].bitcast(i32), in0=qct[:, c0:c0 + CPG],
                in1=y2[:].bitcast(i32), op=SUB)
            nc.vector.tensor_tensor(out=y2[:], in0=y0[:], in1=y0[:], op=MULT)
            nc.vector.scalar_tensor_tensor(
                out=t1[:], in0=ss[grp][:], scalar=-0.5, in1=y2[:],
                op0=MULT, op1=MULT)
            nc.vector.scalar_tensor_tensor(
                out=rr[grp][:], in0=t1[:], scalar=1.5, in1=y0[:],
                op0=ADD, op1=MULT)
            if NEWTON2:
                nc.vector.tensor_tensor(out=y2[:], in0=rr[grp][:],
                                        in1=rr[grp][:], op=MULT)
                nc.vector.scalar_tensor_tensor(
                    out=t1[:], in0=ss[grp][:], scalar=-0.5, in1=y2[:],
                    op0=MULT, op1=MULT)
                nc.vector.scalar_tensor_tensor(
                    out=rr[grp][:], in0=t1[:], scalar=1.5, in1=rr[grp][:],
                    op0=ADD, op1=MULT)
            # scale split DVE/ACT (DVE is the busiest engine), then one DMAT
            if not last:
                for lc in range(CPG):
                    if lc % 2 == 0:
                        nc.vector.tensor_scalar_mul(hn[:, lc, :],
                                                    ttg[:, lc, :],
                                                    rr[grp][:, lc:lc + 1])
                    else:
                        nc.scalar.mul(hn[:, lc, :], ttg[:, lc, :],
                                      rr[grp][:, lc:lc + 1])
                nc.sync.dma_start_transpose(
                    out=hTg[:].rearrange("p c h j -> p (c h) j"),
                    in_=hn[:].rearrange("p c d -> p (c d)"))
            else:
                for lc in range(CPG):
                    nc.vector.tensor_scalar_mul(hf32[:, c0 + lc, :],
                                                ttg[:, lc, :],
                                                rr[grp][:, lc:lc + 1])

        if not SKIP_GATHER:
            raws = {0: alloc_raw()}
            for h in range(4):
                emit_gather_quarter(0, h, raws[0])
            emit_group_precompute(0, raws.pop(0))
            for g in range(GRP):
                for si in range(SPG):
                    s = g * SPG + si
                    if s < NSTEPS:
                        scan_front(s, 0)
                        scan_back(s, 0)
                        scan_front(s, 1)
                        scan_back(s, 1)
                    if g + 1 < GRP:
                        if si == 0:
                            raws[g + 1] = alloc_raw()
                        if si < 4:
                            emit_gather_quarter(g + 1, si, raws[g + 1])
                        if si == 4:
                            emit_group_precompute(g + 1, raws.pop(g + 1))
        else:
            nc.vector.memset(encT[:], 0.0)
            nc.vector.memset(ew_all[:], 0.0)
            nc.vector.memset(ekm[:], 0.0)
            for s in range(NSTEPS):
                scan_front(s, 0)
                scan_back(s, 0)
                scan_front(s, 1)
                scan_back(s, 1)

        if NSTEPS == 0:
            nc.vector.memset(hf32[:], 0.0)
        # ---- output: y[b, k, d] <- hf32[(k%4)*32+b, k//4, d] -------------
        y_main = y_d.ap()[:, 0:28, :].rearrange("b (kh kl) d -> b kl kh d",
                                                kl=4)
        for klo in range(4):
            nc.sync.dma_start(out=y_main[:, klo, :, :],
                              in_=hf32[klo * 32:(klo + 1) * 32, 0:7, :])
        nc.sync.dma_start(out=y_d.ap()[:, 28, :],
                          in_=hf32[0:32, 7, :])
        nc.sync.dma_start(out=y_d.ap()[:, 29, :],
                          in_=hf32[32:64, 7, :])

        for p in reversed(ctxs):
            p.__exit__(None, None, None)

    nc.compile()
    return nc


def _host_prep(prgrph, prgrph_mask, keys, embedding_matrix, U, V, W):
    """Build per-core input maps."""
    prg = np.asarray(prgrph).astype(np.int64)
    msk = np.asarray(prgrph_mask).astype(bool)
    keys = np.asarray(keys, dtype=np.float32)
    embm = np.asarray(embedding_matrix, dtype=np.float32)
    U = np.asarray(U, dtype=np.float32)
    V = np.asarray(V, dtype=np.float32)
    W = np.asarray(W, dtype=np.float32)

    emb_bf = embm.astype(_bf16)

    def halves(m):      # [256, 256] -> [128, 2, 256] bf16
        return np.ascontiguousarray(
            m.reshape(2, 128, D).swapaxes(0, 1).astype(_bf16))

    u_h, v_h, w_h = halves(U), halves(V), halves(W)

    ident = np.eye(128, dtype=_bf16)
    selsum = np.zeros((128, BL), dtype=_bf16)
    p_ar = np.arange(128)
    selsum[p_ar, p_ar % 32] = 1
    selkm = np.ascontiguousarray(selsum.T)
    mdiag = selsum.astype(np.float32)

    # token index layout: flat slot i=q*128+p, p=(l%4)*32+b, q=g*32+s_in*4+l//4
    tok = np.where(msk, prg, VOCAB).astype(np.int64)   # [B, S, L]
    sent_ok = msk.any(-1)                              # [B, S]

    in_maps = []
    for m in range(NCORES):
        b0 = m * BL
        t = tok[b0:b0 + BL]                            # [32, 64, 16]
        # quad dedup: one table row = the 4 l_hi embeddings of (b, s, l_lo)
        quads = t.reshape(BL, S, 4, 4).transpose(0, 1, 3, 2)   # [b, s, l_lo, l_hi]
        qflat = np.ascontiguousarray(quads.reshape(-1, 4))
        uniq, inv = np.unique(qflat, axis=0, return_inverse=True)
        n_u = len(uniq)
        assert n_u <= 8192, f"unique quad overflow: {n_u}"
        emb_core = np.zeros((8192, 4, D), dtype=_bf16)
        safe = np.minimum(uniq, VOCAB)                  # VOCAB -> zero row
        ext = np.vstack([emb_bf, np.zeros((1, D), _bf16)])
        emb_core[:n_u] = ext[safe]
        emb_core = emb_core.reshape(8192, 4 * D)
        inv = inv.reshape(BL, S, 4)                     # [b, s, l_lo]
        # flat slot i = q*128 + p, p = l_lo*32 + b, q = s_in (per group)
        idx = np.zeros((128, 64), dtype=np.int16)       # [p, g*8+s_in]
        s_idx = np.arange(S)
        g_ar, si_ar = s_idx // SPG, s_idx % SPG
        for llo in range(4):
            p = llo * 32 + np.arange(BL)
            q = g_ar * 8 + si_ar
            idx[p[:, None], q[None, :]] = inv[:, :, llo].astype(np.int16)
        # wrap flat order i=q*128+p into [16, n/16] gather layout per group
        cols = []
        for g in range(GRP):
            flat = idx[:, g * 8:(g + 1) * 8].T.reshape(-1)   # i = s_in*128+p
            cols.append(flat.reshape(64, 16).T)
        idx16 = np.ascontiguousarray(np.tile(np.concatenate(cols, axis=1), (8, 1)))
        kT = np.zeros((D, J), dtype=_bf16)
        kloc = np.transpose(keys[b0:b0 + BL], (2, 1, 0))   # [D, K, BL]
        kT[:, :K * BL] = kloc.reshape(D, K * BL)[:, :]
        # j = k*32 + b -> reshape (K, BL) row-major matches k*32+b
        keysT_h = np.ascontiguousarray(kT.reshape(2, 128, J).swapaxes(0, 1))
        bias = np.zeros((128, S), dtype=np.float32)
        ok = sent_ok[b0:b0 + BL]                       # [32, 64]
        bias[:, :] = np.where(ok, 0.0, NEG)[np.arange(128) % 32, :]
        in_maps.append({
            "emb": emb_core, "idx": idx16, "keysT": keysT_h,
            "u": u_h, "v": v_h, "w": w_h, "bias": bias,
            "selsum": selsum, "selkm": selkm, "mdiag": mdiag,
            "ident": ident,
        })
    return in_maps


def kernel(**inputs):
    global LAST_EXEC_NS
    from concourse.bass_utils import run_bass_kernel_spmd

    trace = os.environ.get("BASS_KERNEL_TRACE", "0") == "1"
    if trace:
        try:
            import sys, types

            if "antenv.axon_hooks" not in sys.modules:
                mod = types.ModuleType("antenv.axon_hooks")
                _h = [None]
                mod.set_axon_ntff_profile_hook = lambda h: _h.__setitem__(0, h)
                mod.get_axon_ntff_profile_hook = lambda: _h[0]
                sys.modules["antenv.axon_hooks"] = mod
                import antenv
                antenv.axon_hooks = mod
                from trn_agent_boot.trn_boot import _ntff_profile_via_ctypes
                mod.set_axon_ntff_profile_hook(
                    _ntff_profile_via_ctypes("/opt/axon/libaxon_pjrt.so"))
        except Exception as e:
            print("trace hook unavailable:", e)
            trace = False

    nc = _build_nc()
    in_maps = _host_prep(**inputs)
    res = run_bass_kernel_spmd(nc, in_maps, list(range(NCORES)), trace=trace)
    if trace:
        LAST_EXEC_NS = res.exec_time_ns
    out = np.concatenate([res.results[m]["y"] for m in range(NCORES)], axis=0)
    return out.astype(np.float32)
